# revision 13
# baseline (speedup 1.0000x reference)
"""Trainium2 Bass kernel for nn_Addresser (scatter_memory).

Computation per token t (d_model=2048, H=8 heads, A=D=64):
  addr = softmax_groups64(x @ W_addr + b_addr)   # (t, H, A)
  data = x @ W_data + b_data                     # (t, H, D)
  out[t] = sum_h addr[t,h,:] outer data[t,h,:]   # (t, A*D)

Sharding: data-parallel over the 4096 tokens -> 8 cores x 512 tokens.
Each core runs the same program on its token slice with full weights.

Per-core pipeline (token tiles of 128):
  1. DMA x tile [128, 2048]
  2. PE-transpose into x^T chunks [c=128, t=128] (16 per tile)
  3. Projections on PE (float32r, full rate): PSUM [t, 512] for addr & data,
     bias added via a K=1 ones-outer-product matmul at start of the chain
  4. softmax pieces: exp on ScalarE (no max-sub needed: logits ~N(0,1)),
     grouped reduce + reciprocal + data scaling on VectorE
  5. gather into outer-product operand layouts via SBUF->SBUF DMAs:
       A_g[8g+h, q, a] = exp[16q+g, h, a]            (dense, per-(h,q) DMAs)
       D_st[64z+8g'+h, q, 64g'+d] = data_n[...]      (staircase, per-token DMAs)
  6. outer products on PE: per 16-token group, 2 packed matmuls
     (K=64, M=64, N=512, tile_position row/col groups 0 and 64):
       out[64z+a, 64v+d] = mem[16q+8z+v][a, d]
  7. PSUM -> SBUF copy, DMA to DRAM with per-token strided AP
"""
import sys

sys.path.insert(0, "/opt/trn_rl_repo")

import numpy as np

NUM_CORES = 8
TOK_PER_CORE = 512
D_MODEL = 2048
H, A, D = 8, 64, 64
J = H * A           # 512 projection width
TT = 128            # tokens per tile
NTILE = TOK_PER_CORE // TT
NC_CHUNKS = D_MODEL // 128

_cached = {}


def _build():
    import concourse.bass as bass
    import concourse.mybir as mybir
    import concourse.tile as tile
    from concourse import bacc
    from concourse.masks import make_identity

    dt = mybir.dt
    f32, f32r = dt.float32, dt.float32r

    nc = bacc.Bacc("TRN2", target_bir_lowering=False)
    x_d = nc.dram_tensor("x", [TOK_PER_CORE, D_MODEL], f32, kind="ExternalInput")
    wa_d = nc.dram_tensor("wa", [D_MODEL, J], f32, kind="ExternalInput")
    wd_d = nc.dram_tensor("wd", [D_MODEL, J], f32, kind="ExternalInput")
    ba_d = nc.dram_tensor("ba", [J], f32, kind="ExternalInput")
    bd_d = nc.dram_tensor("bd", [J], f32, kind="ExternalInput")
    out_d = nc.dram_tensor("out", [TOK_PER_CORE, A * D], f32, kind="ExternalOutput")

    with tile.TileContext(nc) as tc:
        _body(tc, bass, mybir, make_identity,
              x_d.ap(), wa_d.ap(), wd_d.ap(), ba_d.ap(), bd_d.ap(), out_d.ap())
    nc.finalize()
    return nc


def _body(tc, bass, mybir, make_identity, x, wa, wd, ba, bd, out):
    dt = mybir.dt
    f32, f32r = dt.float32, dt.float32r
    nc = tc.nc
    import contextlib
    ctx = contextlib.ExitStack()
    singles = ctx.enter_context(tc.tile_pool(name="singles", bufs=1))
    xpool = ctx.enter_context(tc.tile_pool(name="xpool", bufs=2))
    xtpool = ctx.enter_context(tc.tile_pool(name="xtpool", bufs=2))
    smpool = ctx.enter_context(tc.tile_pool(name="smpool", bufs=2))
    gpool = ctx.enter_context(tc.tile_pool(name="gpool", bufs=2))
    ospool = ctx.enter_context(tc.tile_pool(name="ospool", bufs=3))
    pt_ps = ctx.enter_context(tc.tile_pool(name="pt_ps", bufs=2, space="PSUM"))
    pr_ps = ctx.enter_context(tc.tile_pool(name="pr_ps", bufs=2, space="PSUM"))
    op_ps_pool = ctx.enter_context(tc.tile_pool(name="op_ps", bufs=2, space="PSUM"))

    ident = singles.tile([128, 128], f32)
    make_identity(nc, ident)
    ones = singles.tile([1, 128], f32r)
    nc.vector.memset(ones.bitcast(f32), 1.0)

    # resident weights [c_in_chunk(128), chunk(16), j(512)]
    wa_sb = singles.tile([128, NC_CHUNKS, J], f32r, tag="wa_sb")
    nc.sync.dma_start(wa_sb, wa.rearrange("(o p) j -> p o j", p=128).bitcast(f32r))
    wd_sb = singles.tile([128, NC_CHUNKS, J], f32r, tag="wd_sb")
    nc.sync.dma_start(wd_sb, wd.rearrange("(o p) j -> p o j", p=128).bitcast(f32r))
    ba_sb = singles.tile([1, J], f32r, tag="ba_sb")
    nc.sync.dma_start(ba_sb, ba[None, :].bitcast(f32r))
    bd_sb = singles.tile([1, J], f32r, tag="bd_sb")
    nc.sync.dma_start(bd_sb, bd[None, :].bitcast(f32r))

    # combined gather buffers (zeros written once; block slots overwritten in
    # place every use, so the zero background stays valid). Per 16-token group
    # q: free cols [0:128] hold the block-diagonal A region (exp; token g's
    # block at cols 64*(g//8)..+64), cols [128:640] hold the D staircase
    # (scaled data; token g's block at cols 128+64*(g%8)..+64). Token g
    # occupies partitions [8g, 8g+8) (rows (g,h)) in both regions.
    GW = 128 + J  # 640
    gath = [singles.tile([128, 8, GW], f32r, tag=f"gath{i}", name=f"gath{i}")
            for i in range(2)]
    for t_ in gath:
        nc.vector.memset(t_.bitcast(f32), 0.0)

    for tt in range(NTILE):
        t0 = tt * TT
        x_tile = xpool.tile([TT, D_MODEL], f32)
        nc.sync.dma_start(x_tile, x[t0:t0 + TT, :])

        # ---- transpose x tile into [c, t] chunks ----
        xt = xtpool.tile([128, NC_CHUNKS, TT], f32r)
        for k in range(NC_CHUNKS):
            psT = pt_ps.tile([128, TT], f32)
            nc.tensor.transpose(psT, x_tile[:, 128 * k:128 * (k + 1)], ident)
            if k % 2 == 0:
                nc.vector.tensor_copy(xt[:, k, :], psT)
            else:
                nc.scalar.copy(xt[:, k, :], psT)

        # ---- projections ----
        addr_ps = pr_ps.tile([TT, J], f32, tag="addr_ps")
        data_ps = pr_ps.tile([TT, J], f32, tag="data_ps")
        nc.tensor.matmul(addr_ps, ones, ba_sb,
                         start=True, stop=False)
        for k in range(NC_CHUNKS):
            nc.tensor.matmul(addr_ps, xt[:, k, :], wa_sb[:, k, :],
                             start=False, stop=(k == NC_CHUNKS - 1))
        nc.tensor.matmul(data_ps, ones, bd_sb,
                         start=True, stop=False)
        for k in range(NC_CHUNKS):
            nc.tensor.matmul(data_ps, xt[:, k, :], wd_sb[:, k, :],
                             start=False, stop=(k == NC_CHUNKS - 1))

        # ---- softmax pieces (softmax over a within each head group) ----
        # sm[t, h, 0, :] = exp(addr); sm[t, h, 1, :] = data / sum_h
        sm = smpool.tile([TT, H, 2, 64], f32, tag="sm")
        nc.scalar.activation(sm[:, :, 0, :],
                             addr_ps.rearrange("p (h a) -> p h a", h=H),
                             mybir.ActivationFunctionType.Exp)
        sums = smpool.tile([TT, H], f32, tag="sums")
        nc.vector.reduce_sum(sums, sm[:, :, 0, :], axis=mybir.AxisListType.X)
        recip = smpool.tile([TT, H], f32, tag="recip")
        nc.vector.reciprocal(recip, sums)
        # scale data rows by 1/sum of their head (equivalent to scaling addr)
        nc.vector.tensor_tensor(sm[:, :, 1, :],
                                data_ps.rearrange("p (h d) -> p h d", h=H),
                                recip[:, :, None].to_broadcast([TT, H, D]),
                                mybir.AluOpType.mult)

        # ---- gather: one DMA per token fills its A-block rows and staircase
        # block: dest partitions [8g, 8g+8), free chunks {A at 0, D at
        # 64+64*(g%8)}; src = sm[t] rows iterate (h, {exp,data}, 64) to match.
        g_buf = gath[tt % 2]
        for q in range(8):
            for tl in range(16):
                z, g2 = tl // 8, tl % 8
                tok = 16 * q + tl
                a_off = 64 * z
                d_off = 128 + 64 * g2
                dst = bass.AP(tensor=g_buf.tensor,
                              offset=8 * tl * (8 * GW) + q * GW + a_off,
                              ap=[[8 * GW, 8], [d_off - a_off, 2], [1, 64]])
                eng = nc.scalar if (q + tl) % 2 == 0 else nc.sync
                eng.dma_start(dst, sm[tok:tok + 1, :, :, :].bitcast(f32r))

        # ---- outer products: one K=128 block-diagonal matmul per group ----
        for q in range(8):
            op_ps = op_ps_pool.tile([128, J], f32)
            nc.tensor.matmul(op_ps, g_buf[:, q, 0:128], g_buf[:, q, 128:GW],
                             start=True, stop=True)
            os_t = ospool.tile([128, J], f32)
            if q % 2 == 0:
                nc.vector.tensor_copy(os_t, op_ps)
            else:
                nc.scalar.copy(os_t, op_ps)
            # out[t0+16q+8z+v, 64a+d] = os_t[64z+a, 64v+d]
            for z in range(2):
                tok = t0 + 16 * q + 8 * z
                nc.sync.dma_start(
                    out[tok:tok + 8, :].rearrange("v (a d) -> a v d", a=A),
                    os_t[64 * z:64 * z + 64, :].rearrange("a (v d) -> a v d", v=8))
    ctx.close()


def _get_nc():
    if "nc" not in _cached:
        _cached["nc"] = _build()
    return _cached["nc"]


def kernel(tensor, W_addr, b_addr, W_data, b_data):
    from concourse.bass_utils import run_bass_kernel_spmd

    nc = _get_nc()
    X = np.ascontiguousarray(tensor.reshape(-1, D_MODEL)).astype(np.float32)
    wa = np.ascontiguousarray(W_addr, dtype=np.float32)
    wd = np.ascontiguousarray(W_data, dtype=np.float32)
    ba = np.ascontiguousarray(b_addr, dtype=np.float32)
    bd = np.ascontiguousarray(b_data, dtype=np.float32)
    in_maps = []
    for c in range(NUM_CORES):
        sl = X[c * TOK_PER_CORE:(c + 1) * TOK_PER_CORE]
        in_maps.append({"x": np.ascontiguousarray(sl), "wa": wa, "wd": wd,
                        "ba": ba, "bd": bd})
    res = run_bass_kernel_spmd(nc, in_maps, core_ids=list(range(NUM_CORES)))
    _cached["last_results"] = res
    outs = [res.results[c]["out"] for c in range(NUM_CORES)]
    full = np.concatenate(outs, axis=0)
    return full.reshape(*tensor.shape[:-1], A * D)


# revision 15
# speedup vs baseline: 1.0478x; 1.0478x over previous
"""Trainium2 Bass kernel for nn_Addresser (scatter_memory).

Computation per token t (d_model=2048, H=8 heads, A=D=64):
  addr = softmax_groups64(x @ W_addr + b_addr)   # (t, H, A)
  data = x @ W_data + b_data                     # (t, H, D)
  out[t] = sum_h addr[t,h,:] outer data[t,h,:]   # (t, A*D)

Sharding: data-parallel over the 4096 tokens -> 8 cores x 512 tokens.
Each core runs the same program on its token slice with full weights.

Per-core pipeline (token tiles of 128):
  1. DMA x tile [128, 2048]
  2. PE-transpose into x^T chunks [c=128, t=128] (16 per tile)
  3. Projections on PE (float32r, full rate): PSUM [t, 512] for addr & data,
     bias added via a K=1 ones-outer-product matmul at start of the chain
  4. softmax pieces: exp on ScalarE (no max-sub needed: logits ~N(0,1)),
     grouped reduce + reciprocal + data scaling on VectorE
  5. gather into outer-product operand layouts via SBUF->SBUF DMAs:
       A_g[8g+h, q, a] = exp[16q+g, h, a]            (dense, per-(h,q) DMAs)
       D_st[64z+8g'+h, q, 64g'+d] = data_n[...]      (staircase, per-token DMAs)
  6. outer products on PE: per 16-token group, 2 packed matmuls
     (K=64, M=64, N=512, tile_position row/col groups 0 and 64):
       out[64z+a, 64v+d] = mem[16q+8z+v][a, d]
  7. PSUM -> SBUF copy, DMA to DRAM with per-token strided AP
"""
import sys

sys.path.insert(0, "/opt/trn_rl_repo")

import numpy as np

NUM_CORES = 8
TOK_PER_CORE = 512
D_MODEL = 2048
H, A, D = 8, 64, 64
J = H * A           # 512 projection width
TT = 128            # tokens per tile
NTILE = TOK_PER_CORE // TT
NC_CHUNKS = D_MODEL // 128

_cached = {}

import os
ABLATE = set(os.environ.get("KERNEL_ABLATE", "").split(","))


def _build():
    import concourse.bass as bass
    import concourse.mybir as mybir
    import concourse.tile as tile
    from concourse import bacc
    from concourse.masks import make_identity

    dt = mybir.dt
    f32, f32r = dt.float32, dt.float32r

    nc = bacc.Bacc("TRN2", target_bir_lowering=False)
    x_d = nc.dram_tensor("x", [TOK_PER_CORE, D_MODEL], f32, kind="ExternalInput")
    wa_d = nc.dram_tensor("wa", [D_MODEL, J], f32, kind="ExternalInput")
    wd_d = nc.dram_tensor("wd", [D_MODEL, J], f32, kind="ExternalInput")
    ba_d = nc.dram_tensor("ba", [J], f32, kind="ExternalInput")
    bd_d = nc.dram_tensor("bd", [J], f32, kind="ExternalInput")
    out_d = nc.dram_tensor("out", [TOK_PER_CORE, A * D], f32, kind="ExternalOutput")

    with tile.TileContext(nc) as tc:
        _body(tc, bass, mybir, make_identity,
              x_d.ap(), wa_d.ap(), wd_d.ap(), ba_d.ap(), bd_d.ap(), out_d.ap())
    nc.finalize()
    return nc


def _body(tc, bass, mybir, make_identity, x, wa, wd, ba, bd, out):
    dt = mybir.dt
    f32, f32r = dt.float32, dt.float32r
    nc = tc.nc
    import contextlib
    ctx = contextlib.ExitStack()
    singles = ctx.enter_context(tc.tile_pool(name="singles", bufs=1))
    xpool = ctx.enter_context(tc.tile_pool(name="xpool", bufs=2))
    xtpool = ctx.enter_context(tc.tile_pool(name="xtpool", bufs=2))
    smpool = ctx.enter_context(tc.tile_pool(name="smpool", bufs=2))
    gpool = ctx.enter_context(tc.tile_pool(name="gpool", bufs=2))
    ospool = ctx.enter_context(tc.tile_pool(name="ospool", bufs=3))
    pt_ps = ctx.enter_context(tc.tile_pool(name="pt_ps", bufs=2, space="PSUM"))
    pr_ps = ctx.enter_context(tc.tile_pool(name="pr_ps", bufs=2, space="PSUM"))
    op_ps_pool = ctx.enter_context(tc.tile_pool(name="op_ps", bufs=2, space="PSUM"))

    ident = singles.tile([128, 128], f32)
    make_identity(nc, ident)
    ones = singles.tile([1, 128], f32r)
    nc.vector.memset(ones.bitcast(f32), 1.0)

    # resident weights [c_in_chunk(128), chunk(16), j(512)]
    wa_sb = singles.tile([128, NC_CHUNKS, J], f32r, tag="wa_sb")
    nc.sync.dma_start(wa_sb, wa.rearrange("(o p) j -> p o j", p=128).bitcast(f32r))
    wd_sb = singles.tile([128, NC_CHUNKS, J], f32r, tag="wd_sb")
    nc.sync.dma_start(wd_sb, wd.rearrange("(o p) j -> p o j", p=128).bitcast(f32r))
    ba_sb = singles.tile([1, J], f32r, tag="ba_sb")
    nc.sync.dma_start(ba_sb, ba[None, :].bitcast(f32r))
    bd_sb = singles.tile([1, J], f32r, tag="bd_sb")
    nc.sync.dma_start(bd_sb, bd[None, :].bitcast(f32r))

    # combined gather buffers (zeros written once; block slots overwritten in
    # place every use, so the zero background stays valid). Per 16-token group
    # q: free cols [0:128] hold the block-diagonal A region (exp; token g's
    # block at cols 64*(g//8)..+64), cols [128:640] hold the D staircase
    # (scaled data; token g's block at cols 128+64*(g%8)..+64). Token g
    # occupies partitions [8g, 8g+8) (rows (g,h)) in both regions.
    GW = 128 + J  # 640
    gath = [singles.tile([128, 8, GW], f32r, tag=f"gath{i}", name=f"gath{i}")
            for i in range(2)]
    for t_ in gath:
        nc.vector.memset(t_.bitcast(f32), 0.0)

    for tt in range(NTILE):
        t0 = tt * TT
        x_tile = xpool.tile([TT, D_MODEL], f32)
        nc.sync.dma_start(x_tile, x[t0:t0 + TT, :])

        # ---- transpose x tile into [c, t] chunks ----
        xt = xtpool.tile([128, NC_CHUNKS, TT], f32r)
        for k in range(NC_CHUNKS):
            psT = pt_ps.tile([128, TT], f32)
            nc.tensor.transpose(psT, x_tile[:, 128 * k:128 * (k + 1)], ident)
            if k % 2 == 0:
                nc.vector.tensor_copy(xt[:, k, :], psT)
            else:
                nc.scalar.copy(xt[:, k, :], psT)

        # ---- projections ----
        addr_ps = pr_ps.tile([TT, J], f32, tag="addr_ps")
        data_ps = pr_ps.tile([TT, J], f32, tag="data_ps")
        nc.tensor.matmul(addr_ps, ones, ba_sb,
                         start=True, stop=False)
        for k in range(NC_CHUNKS):
            nc.tensor.matmul(addr_ps, xt[:, k, :], wa_sb[:, k, :],
                             start=False, stop=(k == NC_CHUNKS - 1))
        nc.tensor.matmul(data_ps, ones, bd_sb,
                         start=True, stop=False)
        for k in range(NC_CHUNKS):
            nc.tensor.matmul(data_ps, xt[:, k, :], wd_sb[:, k, :],
                             start=False, stop=(k == NC_CHUNKS - 1))

        # ---- softmax pieces (softmax over a within each head group) ----
        # sm[t, h, 0, :] = exp(addr); sm[t, h, 1, :] = data / sum_h
        sm = smpool.tile([TT, H, 2, 64], f32, tag="sm")
        nc.scalar.activation(sm[:, :, 0, :],
                             addr_ps.rearrange("p (h a) -> p h a", h=H),
                             mybir.ActivationFunctionType.Exp)
        sums = smpool.tile([TT, H], f32, tag="sums")
        nc.vector.reduce_sum(sums, sm[:, :, 0, :], axis=mybir.AxisListType.X)
        recip = smpool.tile([TT, H], f32, tag="recip")
        nc.vector.reciprocal(recip, sums)
        # scale data rows by 1/sum of their head (equivalent to scaling addr)
        nc.vector.tensor_tensor(sm[:, :, 1, :],
                                data_ps.rearrange("p (h d) -> p h d", h=H),
                                recip[:, :, None].to_broadcast([TT, H, D]),
                                mybir.AluOpType.mult)

        # ---- gather: one DMA per token fills its A-block rows and staircase
        # block: dest partitions [8g, 8g+8), free chunks {A at 0, D at
        # 64+64*(g%8)}; src = sm[t] rows iterate (h, {exp,data}, 64) to match.
        g_buf = gath[tt % 2]
        for q in range(8):
            if "gather" in ABLATE:
                break
            for tl in range(16):
                z, g2 = tl // 8, tl % 8
                tok = 16 * q + tl
                a_off = 64 * z
                d_off = 128 + 64 * g2
                dst = bass.AP(tensor=g_buf.tensor,
                              offset=8 * tl * (8 * GW) + q * GW + a_off,
                              ap=[[8 * GW, 8], [d_off - a_off, 2], [1, 64]])
                eng = nc.scalar if (q + tl) % 2 == 0 else nc.sync
                eng.dma_start(dst, sm[tok:tok + 1, :, :, :].bitcast(f32r))

        # ---- outer products: one K=128 block-diagonal matmul per group ----
        for q in range(8):
            op_ps = op_ps_pool.tile([128, J], f32)
            nc.tensor.matmul(op_ps, g_buf[:, q, 0:128], g_buf[:, q, 128:GW],
                             start=True, stop=True)
            os_t = ospool.tile([128, J], f32)
            if q % 2 == 0:
                nc.vector.tensor_copy(os_t, op_ps)
            else:
                nc.scalar.copy(os_t, op_ps)
            # out[t0+16q+8z+v, 64a+d] = os_t[64z+a, 64v+d]
            for z in range(2):
                if "outdma" in ABLATE:
                    break
                tok = t0 + 16 * q + 8 * z
                nc.sync.dma_start(
                    out[tok:tok + 8, :].rearrange("v (a d) -> a v d", a=A),
                    os_t[64 * z:64 * z + 64, :].rearrange("a (v d) -> a v d", v=8))
    ctx.close()


def _get_nc():
    if "nc" not in _cached:
        _cached["nc"] = _build()
    return _cached["nc"]


def kernel(tensor, W_addr, b_addr, W_data, b_data):
    from concourse.bass_utils import run_bass_kernel_spmd

    nc = _get_nc()
    X = np.ascontiguousarray(tensor.reshape(-1, D_MODEL)).astype(np.float32)
    wa = np.ascontiguousarray(W_addr, dtype=np.float32)
    wd = np.ascontiguousarray(W_data, dtype=np.float32)
    ba = np.ascontiguousarray(b_addr, dtype=np.float32)
    bd = np.ascontiguousarray(b_data, dtype=np.float32)
    in_maps = []
    for c in range(NUM_CORES):
        sl = X[c * TOK_PER_CORE:(c + 1) * TOK_PER_CORE]
        in_maps.append({"x": np.ascontiguousarray(sl), "wa": wa, "wd": wd,
                        "ba": ba, "bd": bd})
    res = run_bass_kernel_spmd(nc, in_maps, core_ids=list(range(NUM_CORES)))
    _cached["last_results"] = res
    outs = [res.results[c]["out"] for c in range(NUM_CORES)]
    full = np.concatenate(outs, axis=0)
    return full.reshape(*tensor.shape[:-1], A * D)


# revision 16
# speedup vs baseline: 2.1527x; 2.0546x over previous
"""Trainium2 Bass kernel for nn_Addresser (scatter_memory).

Computation per token t (d_model=2048, H=8 heads, A=D=64):
  addr = softmax_groups64(x @ W_addr + b_addr)   # (t, H, A)
  data = x @ W_data + b_data                     # (t, H, D)
  out[t] = sum_h addr[t,h,:] outer data[t,h,:]   # (t, A*D)

Sharding: data-parallel over the 4096 tokens -> 8 cores x 512 tokens.
Each core runs the same program on its token slice with full weights.

Per-core pipeline (token tiles of 128):
  1. DMA x tile [128, 2048]
  2. PE-transpose into x^T chunks [c=128, t=128] (16 per tile)
  3. Projections on PE (float32r, full rate): PSUM [t, 512] for addr & data,
     bias added via a K=1 ones-outer-product matmul at start of the chain
  4. softmax pieces: exp on ScalarE (no max-sub needed: logits ~N(0,1)),
     grouped reduce + reciprocal + data scaling on VectorE
  5. gather into outer-product operand layouts via SBUF->SBUF DMAs:
       A_g[8g+h, q, a] = exp[16q+g, h, a]            (dense, per-(h,q) DMAs)
       D_st[64z+8g'+h, q, 64g'+d] = data_n[...]      (staircase, per-token DMAs)
  6. outer products on PE: per 16-token group, 2 packed matmuls
     (K=64, M=64, N=512, tile_position row/col groups 0 and 64):
       out[64z+a, 64v+d] = mem[16q+8z+v][a, d]
  7. PSUM -> SBUF copy, DMA to DRAM with per-token strided AP
"""
import sys

sys.path.insert(0, "/opt/trn_rl_repo")

import numpy as np

NUM_CORES = 8
TOK_PER_CORE = 512
D_MODEL = 2048
H, A, D = 8, 64, 64
J = H * A           # 512 projection width
TT = 128            # tokens per tile
NTILE = TOK_PER_CORE // TT
NC_CHUNKS = D_MODEL // 128

import os as _os
NTILE_RUN = int(_os.environ.get("KERNEL_NTILE", NTILE))

_cached = {}

import os
ABLATE = set(os.environ.get("KERNEL_ABLATE", "").split(","))


def _build():
    import concourse.bass as bass
    import concourse.mybir as mybir
    import concourse.tile as tile
    from concourse import bacc
    from concourse.masks import make_identity

    dt = mybir.dt
    f32, f32r = dt.float32, dt.float32r

    nc = bacc.Bacc("TRN2", target_bir_lowering=False)
    x_d = nc.dram_tensor("x", [TOK_PER_CORE, D_MODEL], f32, kind="ExternalInput")
    wa_d = nc.dram_tensor("wa", [D_MODEL, J], f32, kind="ExternalInput")
    wd_d = nc.dram_tensor("wd", [D_MODEL, J], f32, kind="ExternalInput")
    ba_d = nc.dram_tensor("ba", [J], f32, kind="ExternalInput")
    bd_d = nc.dram_tensor("bd", [J], f32, kind="ExternalInput")
    out_d = nc.dram_tensor("out", [TOK_PER_CORE, A * D], f32, kind="ExternalOutput")

    with tile.TileContext(nc) as tc:
        _body(tc, bass, mybir, make_identity,
              x_d.ap(), wa_d.ap(), wd_d.ap(), ba_d.ap(), bd_d.ap(), out_d.ap())
    nc.finalize()
    return nc


def _body(tc, bass, mybir, make_identity, x, wa, wd, ba, bd, out):
    dt = mybir.dt
    f32, f32r = dt.float32, dt.float32r
    nc = tc.nc
    import contextlib
    ctx = contextlib.ExitStack()
    singles = ctx.enter_context(tc.tile_pool(name="singles", bufs=1))
    xpool = ctx.enter_context(tc.tile_pool(name="xpool", bufs=2))
    xtpool = ctx.enter_context(tc.tile_pool(name="xtpool", bufs=2))
    smpool = ctx.enter_context(tc.tile_pool(name="smpool", bufs=2))
    gpool = ctx.enter_context(tc.tile_pool(name="gpool", bufs=2))
    ospool = ctx.enter_context(tc.tile_pool(name="ospool", bufs=3))
    pt_ps = ctx.enter_context(tc.tile_pool(name="pt_ps", bufs=2, space="PSUM"))
    pr_ps = ctx.enter_context(tc.tile_pool(name="pr_ps", bufs=2, space="PSUM"))
    op_ps_pool = ctx.enter_context(tc.tile_pool(name="op_ps", bufs=2, space="PSUM"))

    ident = singles.tile([128, 128], f32)
    make_identity(nc, ident)
    ones = singles.tile([1, 128], f32r)
    nc.vector.memset(ones.bitcast(f32), 1.0)

    # resident weights [c_in_chunk(128), chunk(16), j(512)]
    wa_sb = singles.tile([128, NC_CHUNKS, J], f32r, tag="wa_sb")
    nc.sync.dma_start(wa_sb, wa.rearrange("(o p) j -> p o j", p=128).bitcast(f32r))
    wd_sb = singles.tile([128, NC_CHUNKS, J], f32r, tag="wd_sb")
    nc.sync.dma_start(wd_sb, wd.rearrange("(o p) j -> p o j", p=128).bitcast(f32r))
    ba_sb = singles.tile([1, J], f32r, tag="ba_sb")
    nc.sync.dma_start(ba_sb, ba[None, :].bitcast(f32r))
    bd_sb = singles.tile([1, J], f32r, tag="bd_sb")
    nc.sync.dma_start(bd_sb, bd[None, :].bitcast(f32r))

    # combined gather buffers (zeros written once; block slots overwritten in
    # place every use, so the zero background stays valid). Per 16-token group
    # q: free cols [0:128] hold the block-diagonal A region (exp; token g's
    # block at cols 64*(g//8)..+64), cols [128:640] hold the D staircase
    # (scaled data; token g's block at cols 128+64*(g%8)..+64). Token g
    # occupies partitions [8g, 8g+8) (rows (g,h)) in both regions.
    GW = 128 + J  # 640
    gath = [singles.tile([128, 8, GW], f32r, tag=f"gath{i}", name=f"gath{i}")
            for i in range(2)]
    for t_ in gath:
        nc.vector.memset(t_.bitcast(f32), 0.0)

    for tt in range(NTILE_RUN):
        t0 = tt * TT
        x_tile = xpool.tile([TT, D_MODEL], f32)
        nc.sync.dma_start(x_tile, x[t0:t0 + TT, :])

        # ---- transpose x tile into [c, t] chunks ----
        xt = xtpool.tile([128, NC_CHUNKS, TT], f32r)
        for k in range(NC_CHUNKS):
            psT = pt_ps.tile([128, TT], f32)
            nc.tensor.transpose(psT, x_tile[:, 128 * k:128 * (k + 1)], ident)
            if k % 2 == 0:
                nc.vector.tensor_copy(xt[:, k, :], psT)
            else:
                nc.scalar.copy(xt[:, k, :], psT)

        # ---- projections ----
        addr_ps = pr_ps.tile([TT, J], f32, tag="addr_ps")
        data_ps = pr_ps.tile([TT, J], f32, tag="data_ps")
        nc.tensor.matmul(addr_ps, ones, ba_sb,
                         start=True, stop=False)
        for k in range(NC_CHUNKS):
            nc.tensor.matmul(addr_ps, xt[:, k, :], wa_sb[:, k, :],
                             start=False, stop=(k == NC_CHUNKS - 1))
        nc.tensor.matmul(data_ps, ones, bd_sb,
                         start=True, stop=False)
        for k in range(NC_CHUNKS):
            nc.tensor.matmul(data_ps, xt[:, k, :], wd_sb[:, k, :],
                             start=False, stop=(k == NC_CHUNKS - 1))

        # ---- softmax pieces (softmax over a within each head group) ----
        # sm[t, h, 0, :] = exp(addr); sm[t, h, 1, :] = data / sum_h
        sm = smpool.tile([TT, H, 2, 64], f32, tag="sm")
        nc.scalar.activation(sm[:, :, 0, :],
                             addr_ps.rearrange("p (h a) -> p h a", h=H),
                             mybir.ActivationFunctionType.Exp)
        sums = smpool.tile([TT, H], f32, tag="sums")
        nc.vector.reduce_sum(sums, sm[:, :, 0, :], axis=mybir.AxisListType.X)
        recip = smpool.tile([TT, H], f32, tag="recip")
        nc.vector.reciprocal(recip, sums)
        # scale data rows by 1/sum of their head (equivalent to scaling addr)
        nc.vector.tensor_tensor(sm[:, :, 1, :],
                                data_ps.rearrange("p (h d) -> p h d", h=H),
                                recip[:, :, None].to_broadcast([TT, H, D]),
                                mybir.AluOpType.mult)

        # ---- gather: one DMA per token fills its A-block rows and staircase
        # block: dest partitions [8g, 8g+8), free chunks {A at 0, D at
        # 64+64*(g%8)}; src = sm[t] rows iterate (h, {exp,data}, 64) to match.
        g_buf = gath[tt % 2]
        for q in range(8):
            if "gather" in ABLATE:
                break
            for tl in range(16):
                z, g2 = tl // 8, tl % 8
                tok = 16 * q + tl
                a_off = 64 * z
                d_off = 128 + 64 * g2
                dst = bass.AP(tensor=g_buf.tensor,
                              offset=8 * tl * (8 * GW) + q * GW + a_off,
                              ap=[[8 * GW, 8], [d_off - a_off, 2], [1, 64]])
                eng = nc.scalar if (q + tl) % 2 == 0 else nc.sync
                eng.dma_start(dst, sm[tok:tok + 1, :, :, :].bitcast(f32r))

        # ---- outer products: one K=128 block-diagonal matmul per group ----
        for q in range(8):
            op_ps = op_ps_pool.tile([128, J], f32)
            nc.tensor.matmul(op_ps, g_buf[:, q, 0:128], g_buf[:, q, 128:GW],
                             start=True, stop=True)
            os_t = ospool.tile([128, J], f32)
            if q % 2 == 0:
                nc.vector.tensor_copy(os_t, op_ps)
            else:
                nc.scalar.copy(os_t, op_ps)
            # out[t0+16q+8z+v, 64a+d] = os_t[64z+a, 64v+d]
            for z in range(2):
                if "outdma" in ABLATE:
                    break
                tok = t0 + 16 * q + 8 * z
                nc.sync.dma_start(
                    out[tok:tok + 8, :].rearrange("v (a d) -> a v d", a=A),
                    os_t[64 * z:64 * z + 64, :].rearrange("a (v d) -> a v d", v=8))
    ctx.close()


def _get_nc():
    if "nc" not in _cached:
        _cached["nc"] = _build()
    return _cached["nc"]


def kernel(tensor, W_addr, b_addr, W_data, b_data):
    from concourse.bass_utils import run_bass_kernel_spmd

    nc = _get_nc()
    X = np.ascontiguousarray(tensor.reshape(-1, D_MODEL)).astype(np.float32)
    wa = np.ascontiguousarray(W_addr, dtype=np.float32)
    wd = np.ascontiguousarray(W_data, dtype=np.float32)
    ba = np.ascontiguousarray(b_addr, dtype=np.float32)
    bd = np.ascontiguousarray(b_data, dtype=np.float32)
    in_maps = []
    for c in range(NUM_CORES):
        sl = X[c * TOK_PER_CORE:(c + 1) * TOK_PER_CORE]
        in_maps.append({"x": np.ascontiguousarray(sl), "wa": wa, "wd": wd,
                        "ba": ba, "bd": bd})
    res = run_bass_kernel_spmd(nc, in_maps, core_ids=list(range(NUM_CORES)))
    _cached["last_results"] = res
    outs = [res.results[c]["out"] for c in range(NUM_CORES)]
    full = np.concatenate(outs, axis=0)
    return full.reshape(*tensor.shape[:-1], A * D)
